# revision 41
# baseline (speedup 1.0000x reference)
"""Trainium2 Bass kernel for nn_AutoCorrelation (8 NeuronCores, data-parallel over batch).

Algorithm (reference: AutoCorrelation block):
  corr = irfft(rfft(q, L) * conj(rfft(k, L)))        # circular cross-correlation
  top-6 delays from batch-mean of corr (mean over H,E then N)
  out  = sum_k softmax(mean[:, idx])_k * roll(v, -idx_k)

Implementation:
  - FFTs become dense DFT matmuls on the TensorEngine: rfft -> q @ C and
    q @ Sm with C[l,f]=cos(2*pi*l*f/L), Sm[l,f]=-sin(...), f=0..511, and the
    Nyquist (f=512) cosine column packed into Sm[:,0] (sin column there is 0).
    irfft -> Pre @ A + Pim @ B with the matching inverse rows (A[0]=DC row,
    B[0]=Nyquist row).
  - Phase 1 kernel (per core, 4 batch items): forward DFTs, pointwise complex
    product (VectorE), inverse DFT, corr written to HBM, per-delay row-sums
    reduced for the top-k statistic.
  - Host: tiny (N,L) mean -> top-6 indices + softmax weights.
  - Phase 2 kernel: out = sum_k w*roll(v) as PSUM-accumulated matmuls with
    w-scaled shifted-identity stationary matrices (shift along L = partition
    permutation, contracted on the TensorEngine).
"""
import math
import sys

sys.path.insert(0, "/opt/trn_rl_repo")

import numpy as np
import ml_dtypes

import concourse.bass as bass
import concourse.tile as tile
from concourse import bacc, mybir
from concourse.bass import ts
from concourse.bass_utils import run_bass_kernel_spmd

_dt = mybir.dt

N, L, H, E = 32, 1024, 8, 64
R = H * E                 # 512 rows (h,e) per batch item
NCORES = 8
NLOC = N // NCORES        # 4 batch items per core
F = 512                   # packed rfft freqs (f=0..511; Nyquist in slot 0)
F2 = 256                  # freqs per radix-2 half (even / odd)
TOPK = int(1.0 * math.log(L))  # 6
LB = L // 128             # 8 l/tau blocks
FB = F // 128             # 4 f blocks
HB = 4                    # 128-blocks per 512-half

# phase-1 matmul dtype: "f32r" (full fp32 precision at ~bf16 rate) or "bf16"
P1_MODE = "bf16"
TRACE = [False]           # test.py flips this to collect exec_time_ns
LAST_EXEC_NS = [0, 0]     # phase1, phase2 exec time (when TRACE)


def _dft_mats():
    """Radix-2 split matrices. Forward (contract over l' = 0..511):
    even freqs X[2m] = (x1+x2) @ [C5 | S5m] (S5m slot 0 = f=512 Nyquist),
    odd freqs X[2m+1] = (x1-x2) @ [Mre | Mim] (twiddle folded in).
    Inverse: u = Pe_re@Au + Pe_im@Bu, w = Po_re@Aw + Po_im@Bw,
    corr[t] = u+w, corr[t+512] = u-w."""
    l = np.arange(512)[:, None].astype(np.float64)
    m = np.arange(F2)[None, :].astype(np.float64)
    C5 = np.cos(2 * np.pi * l * m / 512)
    S5 = -np.sin(2 * np.pi * l * m / 512)
    S5[:, 0] = (-1.0) ** np.arange(512)
    Mre = np.cos(2 * np.pi * l * (2 * m + 1) / L)
    Mim = -np.sin(2 * np.pi * l * (2 * m + 1) / L)
    t = np.arange(512)[None, :].astype(np.float64)
    mm = np.arange(F2)[:, None].astype(np.float64)
    Au = (2.0 / L) * np.cos(2 * np.pi * mm * t / 512)
    Bu = -(2.0 / L) * np.sin(2 * np.pi * mm * t / 512)
    Au[0, :] = 1.0 / L
    Bu[0, :] = (1.0 / L) * ((-1.0) ** np.arange(512))
    Aw = (2.0 / L) * np.cos(2 * np.pi * t * (2 * mm + 1) / L)
    Bw = -(2.0 / L) * np.sin(2 * np.pi * t * (2 * mm + 1) / L)
    return C5, S5, Mre, Mim, Au, Bu, Aw, Bw


def _build_phase1(mode):
    store = _dt.bfloat16

    nc = bacc.Bacc("TRN2", target_bir_lowering=False, debug=False,
                   num_devices=NCORES)
    # q/k are host pre-shuffled to [NLOC, 128, LB*R]: partition line p holds
    # blocks a=0..7 (x[a*128+p, :]) contiguously -> 1 descriptor per line.
    q_d = nc.dram_tensor("q", [NLOC, 128, LB * R], store,
                         kind="ExternalInput").ap()
    k_d = nc.dram_tensor("k", [NLOC, 128, LB * R], store,
                         kind="ExternalInput").ap()
    cst_d = {}
    for nm in ("c5", "s5", "mre", "mim"):
        cst_d[nm] = nc.dram_tensor(nm, [512, F2], store,
                                   kind="ExternalInput").ap()
    for nm in ("au", "bu", "aw", "bw"):
        cst_d[nm] = nc.dram_tensor(nm, [F2, 512], store,
                                   kind="ExternalInput").ap()
    # u/w halves of the inverse DFT (cols 0..3 = u blocks, 4..7 = w blocks,
    # partition-line layout); host does the final u+-w combine.
    uw_d = nc.dram_tensor("uw", [NLOC, 128, LB * R], _dt.bfloat16,
                          kind="ExternalOutput").ap()
    # per-group row-sums of P: cols 0..3 = re (E0,E1,O0,O1), 4..7 = im
    pacc_d = nc.dram_tensor("pacc", [NLOC, 128, 8], _dt.float32,
                            kind="ExternalOutput").ap()

    def mm(ps, lhsT, rhs, start, stop):
        nc.tensor.matmul(ps, lhsT, rhs, start=start, stop=stop)

    with tile.TileContext(nc) as tc:
        with tc.tile_pool(name="const", bufs=1) as cp, \
             tc.tile_pool(name="qk", bufs=4) as qk, \
             tc.tile_pool(name="ed", bufs=8) as edp, \
             tc.tile_pool(name="pp", bufs=8) as pp, \
             tc.tile_pool(name="tmp", bufs=3) as tp, \
             tc.tile_pool(name="out", bufs=2) as op, \
             tc.tile_pool(name="acc", bufs=5) as accp, \
             tc.tile_pool(name="ps", bufs=4, space="PSUM") as psf, \
             tc.tile_pool(name="psi", bufs=4, space="PSUM") as psi:

            # Batched const loads: one DMA trigger per matrix.
            # fwd mats [512, F2] -> [128, 4*F2]; inv mats [F2, 512] -> [128, 2*512]
            # All input loads issued UPFRONT so no load trigger ever queues
            # behind a store trigger waiting on compute (head-of-line block).
            cmats = {}
            qts, kts = [], []
            q0 = qk.tile([128, LB * R], store, tag="q")
            nc.sync.dma_start(q0[:], q_d[0])
            k0 = qk.tile([128, LB * R], store, tag="k")
            nc.scalar.dma_start(k0[:], k_d[0])
            qts.append(q0)
            kts.append(k0)
            for i, nm in enumerate(("c5", "s5", "mre", "mim")):
                t = cp.tile([128, HB * F2], store, tag=nm)
                (nc.sync if i % 2 else nc.scalar).dma_start(
                    t[:], cst_d[nm].rearrange("(j p) c -> p j c", p=128))
                cmats[nm] = t
            for i, nm in enumerate(("au", "bu", "aw", "bw")):
                t = cp.tile([128, 2 * 512], store, tag=nm)
                (nc.scalar if i % 2 else nc.sync).dma_start(
                    t[:], cst_d[nm].rearrange("(g p) c -> p g c", p=128))
                cmats[nm] = t
            for n in range(1, NLOC):
                t = qk.tile([128, LB * R], store, tag="q")
                nc.sync.dma_start(t[:], q_d[n])
                qts.append(t)
                t = qk.tile([128, LB * R], store, tag="k")
                nc.scalar.dma_start(t[:], k_d[n])
                kts.append(t)

            def cslice(nm, j, mb, w):
                # fwd stationary block j: rows 128 (contraction), 128 cols
                # out of the per-block width w (128 or F2)
                return cmats[nm][:, j * w + mb * 128:j * w + mb * 128 + 128]

            def islice(nm, gb, tb):
                return cmats[nm][:, gb * 512 + tb * 128:gb * 512 + tb * 128 + 128]

            for n in range(NLOC):
                qt, kt = qts[n], kts[n]

                # radix-2 butterflies: one [128, R] op per (tensor, j) block,
                # DVE (2x bf16) takes 10, GpSimd 6.
                eq, dq, ek, dk = [], [], [], []
                plan = (("eq", eq, qt, "tensor_add", (0, 0, 1, 0)),
                        ("dq", dq, qt, "tensor_sub", (0, 1, 0, 1)),
                        ("ek", ek, kt, "tensor_add", (0, 0, 0, 0)),
                        ("dk", dk, kt, "tensor_sub", (1, 1, 1, 1)))
                for tag, lst, xt, fn, eng in plan:
                    for j in range(HB):
                        t = edp.tile([128, R], store, tag=tag)
                        e = nc.gpsimd if eng[j] else nc.vector
                        getattr(e, fn)(t[:], xt[:, ts(j, R)],
                                       xt[:, ts(j + 4, R)])
                        lst.append(t)

                acc = accp.tile([128, 8], _dt.float32, tag="acc")
                pre_sb, pim_sb = [], []
                groups = [("c5", "s5", eq, ek, 0, F2, HB),
                          ("c5", "s5", eq, ek, 1, F2, HB),
                          ("mre", "mim", dq, dk, 0, F2, HB),
                          ("mre", "mim", dq, dk, 1, F2, HB)]
                for gi, (ma, mb_, xq, xk, mb, cw, nj) in enumerate(groups):
                    ps_qre = psf.tile([128, R], _dt.float32, tag="fwd")
                    ps_qim = psf.tile([128, R], _dt.float32, tag="fwd")
                    ps_kre = psf.tile([128, R], _dt.float32, tag="fwd")
                    ps_kim = psf.tile([128, R], _dt.float32, tag="fwd")
                    for j in range(nj):
                        mm(ps_qre[:], cslice(ma, j, mb, cw), xq[j][:],
                           j == 0, j == nj - 1)
                    for j in range(nj):
                        mm(ps_kre[:], cslice(ma, j, mb, cw), xk[j][:],
                           j == 0, j == nj - 1)
                    for j in range(nj):
                        mm(ps_qim[:], cslice(mb_, j, mb, cw), xq[j][:],
                           j == 0, j == nj - 1)
                    for j in range(nj):
                        mm(ps_kim[:], cslice(mb_, j, mb, cw), xk[j][:],
                           j == 0, j == nj - 1)

                    # stage Q/K to bf16 SBUF (DVE 2x mode for the muls);
                    # copies split DVE/ACT to balance engine load
                    qre = tp.tile([128, R], store, tag="qre")
                    qim = tp.tile([128, R], store, tag="qim")
                    kre = tp.tile([128, R], store, tag="kre")
                    kim = tp.tile([128, R], store, tag="kim")
                    nc.scalar.mul(qre[:], ps_qre[:], 1.0)
                    nc.scalar.mul(qim[:], ps_qim[:], 1.0)
                    nc.scalar.mul(kre[:], ps_kre[:], 1.0)
                    nc.scalar.mul(kim[:], ps_kim[:], 1.0)
                    t1 = tp.tile([128, R], store, tag="t1")
                    t2 = tp.tile([128, R], store, tag="t2")
                    nc.vector.tensor_mul(t1[:], qre[:], kre[:])
                    nc.vector.tensor_mul(t2[:], qim[:], kim[:])
                    pre = pp.tile([128, R], store, tag="pre")
                    nc.vector.scalar_tensor_tensor(
                        pre[:], t1[:], 1.0, t2[:],
                        op0=mybir.AluOpType.mult, op1=mybir.AluOpType.add,
                        accum_out=acc[:, gi:gi + 1])
                    t3 = tp.tile([128, R], store, tag="t3")
                    t4 = tp.tile([128, R], store, tag="t4")
                    nc.vector.tensor_mul(t3[:], qim[:], kre[:])
                    nc.vector.tensor_mul(t4[:], qre[:], kim[:])
                    pim = pp.tile([128, R], store, tag="pim")
                    nc.vector.scalar_tensor_tensor(
                        pim[:], t3[:], 1.0, t4[:],
                        op0=mybir.AluOpType.mult, op1=mybir.AluOpType.subtract,
                        accum_out=acc[:, 4 + gi:5 + gi])
                    if gi == 0:
                        # slot 0 packs DC (re) / Nyquist (im): overwrite with
                        # pure products and patch the two accum elements
                        nc.vector.tensor_copy(pre[0:1, :], t1[0:1, :])
                        nc.vector.tensor_copy(pim[0:1, :], t2[0:1, :])
                        nc.vector.tensor_reduce(
                            acc[0:1, 0:1], t1[0:1, :],
                            axis=mybir.AxisListType.X, op=mybir.AluOpType.add)
                        nc.vector.tensor_reduce(
                            acc[0:1, 4:5], t2[0:1, :],
                            axis=mybir.AxisListType.X, op=mybir.AluOpType.add)
                    pre_sb.append(pre)
                    pim_sb.append(pim)

                uw = op.tile([128, LB * R], store, tag="uw")
                for tb in range(HB):
                    ps_u = psi.tile([128, R], _dt.float32, tag="inv")
                    ps_w = psi.tile([128, R], _dt.float32, tag="inv")
                    for gb in range(2):
                        mm(ps_u[:], islice("au", gb, tb),
                           pre_sb[gb][:], gb == 0, False)
                        mm(ps_u[:], islice("bu", gb, tb),
                           pim_sb[gb][:], False, gb == 1)
                    for gb in range(2):
                        mm(ps_w[:], islice("aw", gb, tb),
                           pre_sb[2 + gb][:], gb == 0, False)
                        mm(ps_w[:], islice("bw", gb, tb),
                           pim_sb[2 + gb][:], False, gb == 1)
                    nc.scalar.mul(uw[:, ts(tb, R)], ps_u[:], 1.0)
                    nc.scalar.mul(uw[:, ts(tb + HB, R)], ps_w[:], 1.0)
                nc.sync.dma_start(uw_d[n], uw[:])
                nc.scalar.dma_start(pacc_d[n][:], acc[:])
    nc.compile()
    return nc

def _build_phase2(entries):
    """entries: per output block b, list of (src_block, seg_idx); seg_idx
    indexes the g stationaries tensor (NLOC, NSEG, 128, 128)."""
    nseg = max(si for segs in entries for _, si in segs) + 1
    nc = bacc.Bacc("TRN2", target_bir_lowering=False, debug=False,
                   num_devices=NCORES)
    # v host pre-shuffled to [NLOC, 128, LB*R] (block a at cols a*R).
    v_d = nc.dram_tensor("v", [NLOC, 128, LB * R], _dt.bfloat16,
                         kind="ExternalInput").ap()
    # g is host-packed as (NLOC, 128, nseg*128): one contiguous DMA per n;
    # stationary si is the [:, si*128:(si+1)*128] slice.
    g_d = nc.dram_tensor("g", [NLOC, 128, nseg * 128], _dt.bfloat16,
                         kind="ExternalInput").ap()
    out_d = nc.dram_tensor("out", [NLOC, 128, LB * R], _dt.bfloat16,
                           kind="ExternalOutput").ap()

    with tile.TileContext(nc) as tc:
        with tc.tile_pool(name="v", bufs=NLOC) as vp, \
             tc.tile_pool(name="g", bufs=NLOC) as gp, \
             tc.tile_pool(name="o", bufs=3) as op, \
             tc.tile_pool(name="ps", bufs=8, space="PSUM") as psp:
            # One batched DMA per tensor: v[n] -> [128, LB*R], g[n] whole.
            v_sb, g_sb = [], []
            for n in range(NLOC):
                t = vp.tile([128, LB * R], _dt.bfloat16, tag="v")
                (nc.scalar if n % 2 else nc.sync).dma_start(t[:], v_d[n])
                v_sb.append(t)
                tg = gp.tile([128, nseg * 128], _dt.bfloat16, tag="g")
                (nc.sync if n % 2 else nc.scalar).dma_start(tg[:], g_d[n][:])
                g_sb.append(tg)
            for n in range(NLOC):
                ot = op.tile([128, LB * R], _dt.bfloat16, tag="o")
                for b in range(LB):
                    segs = entries[b]
                    ps = psp.tile([128, R], _dt.float32, tag="ps")
                    for i, (a, si) in enumerate(segs):
                        nc.tensor.matmul(ps[:], g_sb[n][:, ts(si, 128)],
                                         v_sb[n][:, ts(a, R)],
                                         start=(i == 0),
                                         stop=(i == len(segs) - 1))
                    if b % 2:
                        nc.scalar.mul(ot[:, ts(b, R)], ps[:], 1.0)
                    else:
                        nc.vector.tensor_copy(ot[:, ts(b, R)], ps[:])
                nc.sync.dma_start(out_d[n], ot[:])
    nc.compile()
    return nc


_P1_CACHE = {}


def _phase1_nc(mode):
    if mode not in _P1_CACHE:
        _P1_CACHE[mode] = _build_phase1(mode)
    return _P1_CACHE[mode]


def _run(nc, in_maps, phase):
    res = run_bass_kernel_spmd(nc, in_maps, core_ids=list(range(NCORES)),
                               trace=TRACE[0])
    if TRACE[0]:
        LAST_EXEC_NS[phase] = res.exec_time_ns
    return res.results


def kernel(queries, keys, values):
    queries = np.ascontiguousarray(np.asarray(queries, dtype=np.float32))
    keys = np.ascontiguousarray(np.asarray(keys, dtype=np.float32))
    values = np.ascontiguousarray(np.asarray(values, dtype=np.float32))

    mode = P1_MODE
    store_np = ml_dtypes.bfloat16
    C5, S5, Mre, Mim, Au, Bu, Aw, Bw = _dft_mats()
    consts = {
        "c5": C5, "s5": S5, "mre": Mre, "mim": Mim,
        "au": Au, "bu": Bu, "aw": Aw, "bw": Bw,
    }
    consts = {k: np.ascontiguousarray(v.astype(np.float32)).astype(store_np)
              for k, v in consts.items()}

    # shuffle to [N, 128, LB, R]: SBUF partition line p = blocks a contiguous
    def shuf(x):
        return np.ascontiguousarray(
            x.reshape(N, LB, 128, R).transpose(0, 2, 1, 3)
        ).reshape(N, 128, LB * R)

    q3 = shuf(queries.reshape(N, L, R).astype(store_np))
    k3 = shuf(keys.reshape(N, L, R).astype(store_np))
    v3 = shuf(values.reshape(N, L, R).astype(ml_dtypes.bfloat16))

    nc1 = _phase1_nc(mode)
    in_maps = []
    for c in range(NCORES):
        sl = slice(c * NLOC, (c + 1) * NLOC)
        in_maps.append({
            "q": q3[sl],
            "k": k3[sl],
            **consts,
        })
    res1 = _run(nc1, in_maps, 0)

    uw = np.concatenate([np.asarray(r["uw"], np.float32) for r in res1],
                        axis=0)                        # (N, 128, 8*R)
    uw = uw.reshape(N, 128, LB, R).transpose(0, 2, 1, 3)  # (N, 8, 128, R)
    u = uw[:, 0:HB].reshape(N, 512, R)
    wv = uw[:, HB:LB].reshape(N, 512, R)
    corr = np.concatenate([u + wv, u - wv], axis=1)           # (N, L, R) f32
    pacc = np.concatenate([r["pacc"] for r in res1], axis=0)  # (N, 128, 8)
    # reconstruct mean over (H,E) from per-group P row-sums (host irfft on
    # a 512-vector per batch item)
    pacc = pacc.astype(np.float64)
    per_ = pacc[:, :, 0:2].transpose(0, 2, 1).reshape(N, 256)   # Pe_re sums
    por_ = pacc[:, :, 2:4].transpose(0, 2, 1).reshape(N, 256)   # Po_re
    pei_ = pacc[:, :, 4:6].transpose(0, 2, 1).reshape(N, 256)   # Pe_im
    poi_ = pacc[:, :, 6:8].transpose(0, 2, 1).reshape(N, 256)   # Po_im
    um = per_ @ Au + pei_ @ Bu
    wm = por_ @ Aw + poi_ @ Bw
    mean = np.concatenate([um + wm, um - wm], axis=1) / R       # (N, L)

    g = mean.mean(axis=0)
    idx = np.argsort(-g, kind="stable")[:TOPK]
    w = mean[:, idx]
    e = np.exp(w - w.max(axis=1, keepdims=True))
    w = (e / e.sum(axis=1, keepdims=True)).astype(np.float32)  # (N, TOPK)

    # phase-2 stationaries: out[b*128+j] += w_k * v[(b*128+j+idx_k) mod L]
    # merged per (b, src_block); matrix content is b-independent, so dedup
    # identical segment sets across b.
    seg_of = {}
    pat = []
    entries = [[] for _ in range(LB)]
    for b in range(LB):
        acc = {}
        for kk in range(TOPK):
            sh = int(idx[kk])
            r = sh % 128
            a = ((b * 128 + sh) // 128) % LB
            acc.setdefault(a, []).append(("d1", r, kk))
            if r > 0:
                acc.setdefault((a + 1) % LB, []).append(("d2", r, kk))
        for a, parts in sorted(acc.items()):
            key = tuple(sorted(parts))
            if key not in seg_of:
                seg_of[key] = len(pat)
                pat.append(parts)
            entries[b].append((a, seg_of[key]))
    nseg = len(pat)
    gmat = np.zeros((NLOC * NCORES, nseg, 128, 128), np.float32)
    jj = np.arange(128)
    for si, parts in enumerate(pat):
        for which, r, kk in parts:
            if which == "d1":
                j = jj[: 128 - r]
                gmat[:, si, j + r, j] += w[:, kk][:, None]
            else:
                j = jj[128 - r:]
                gmat[:, si, j - (128 - r), j] += w[:, kk][:, None]
    # pack (NLOC, nseg, 128, 128) -> (NLOC, 128, nseg*128) for 1-DMA-per-n
    gmat = np.ascontiguousarray(
        gmat.transpose(0, 2, 1, 3).reshape(NLOC * NCORES, 128, nseg * 128)
    ).astype(ml_dtypes.bfloat16)

    nc2 = _build_phase2(entries)
    in_maps2 = []
    for c in range(NCORES):
        sl = slice(c * NLOC, (c + 1) * NLOC)
        in_maps2.append({
            "v": v3[sl],
            "g": gmat[sl],
        })
    res2 = _run(nc2, in_maps2, 1)
    out = np.concatenate([np.asarray(r["out"], dtype=np.float32)
                          for r in res2], axis=0)   # (N, 128, LB*R) shuffled
    # un-shuffle: [N, 128, LB, R] -> [N, LB, 128, R] -> (N, L, R)
    out = out.reshape(N, 128, LB, R).transpose(0, 2, 1, 3).reshape(N, L, R)

    out_full = np.ascontiguousarray(out).reshape(N, L, H, E)
    corr_full = corr.reshape(N, L, H, E).astype(np.float32)
    return out_full, corr_full



# revision 43
# speedup vs baseline: 1.0187x; 1.0187x over previous
"""Trainium2 Bass kernel for nn_AutoCorrelation (8 NeuronCores, data-parallel over batch).

Algorithm (reference: AutoCorrelation block):
  corr = irfft(rfft(q, L) * conj(rfft(k, L)))        # circular cross-correlation
  top-6 delays from batch-mean of corr (mean over H,E then N)
  out  = sum_k softmax(mean[:, idx])_k * roll(v, -idx_k)

Implementation:
  - FFTs become dense DFT matmuls on the TensorEngine: rfft -> q @ C and
    q @ Sm with C[l,f]=cos(2*pi*l*f/L), Sm[l,f]=-sin(...), f=0..511, and the
    Nyquist (f=512) cosine column packed into Sm[:,0] (sin column there is 0).
    irfft -> Pre @ A + Pim @ B with the matching inverse rows (A[0]=DC row,
    B[0]=Nyquist row).
  - Phase 1 kernel (per core, 4 batch items): forward DFTs, pointwise complex
    product (VectorE), inverse DFT, corr written to HBM, per-delay row-sums
    reduced for the top-k statistic.
  - Host: tiny (N,L) mean -> top-6 indices + softmax weights.
  - Phase 2 kernel: out = sum_k w*roll(v) as PSUM-accumulated matmuls with
    w-scaled shifted-identity stationary matrices (shift along L = partition
    permutation, contracted on the TensorEngine).
"""
import math
import sys

sys.path.insert(0, "/opt/trn_rl_repo")

import numpy as np
import ml_dtypes

import concourse.bass as bass
import concourse.tile as tile
from concourse import bacc, mybir
from concourse.bass import ts
from concourse.bass_utils import run_bass_kernel_spmd

_dt = mybir.dt

N, L, H, E = 32, 1024, 8, 64
R = H * E                 # 512 rows (h,e) per batch item
NCORES = 8
NLOC = N // NCORES        # 4 batch items per core
F = 512                   # packed rfft freqs (f=0..511; Nyquist in slot 0)
F2 = 256                  # freqs per radix-2 half (even / odd)
TOPK = int(1.0 * math.log(L))  # 6
LB = L // 128             # 8 l/tau blocks
FB = F // 128             # 4 f blocks
HB = 4                    # 128-blocks per 512-half

# phase-1 matmul dtype: "f32r" (full fp32 precision at ~bf16 rate) or "bf16"
P1_MODE = "bf16"
TRACE = [False]           # test.py flips this to collect exec_time_ns
LAST_EXEC_NS = [0, 0]     # phase1, phase2 exec time (when TRACE)


def _dft_mats():
    """Radix-2 split matrices. Forward (contract over l' = 0..511):
    even freqs X[2m] = (x1+x2) @ [C5 | S5m] (S5m slot 0 = f=512 Nyquist),
    odd freqs X[2m+1] = (x1-x2) @ [Mre | Mim] (twiddle folded in).
    Inverse: u = Pe_re@Au + Pe_im@Bu, w = Po_re@Aw + Po_im@Bw,
    corr[t] = u+w, corr[t+512] = u-w."""
    l = np.arange(512)[:, None].astype(np.float64)
    m = np.arange(F2)[None, :].astype(np.float64)
    C5 = np.cos(2 * np.pi * l * m / 512)
    S5 = -np.sin(2 * np.pi * l * m / 512)
    S5[:, 0] = (-1.0) ** np.arange(512)
    Mre = np.cos(2 * np.pi * l * (2 * m + 1) / L)
    Mim = -np.sin(2 * np.pi * l * (2 * m + 1) / L)
    t = np.arange(512)[None, :].astype(np.float64)
    mm = np.arange(F2)[:, None].astype(np.float64)
    Au = (2.0 / L) * np.cos(2 * np.pi * mm * t / 512)
    Bu = -(2.0 / L) * np.sin(2 * np.pi * mm * t / 512)
    Au[0, :] = 1.0 / L
    Bu[0, :] = (1.0 / L) * ((-1.0) ** np.arange(512))
    Aw = (2.0 / L) * np.cos(2 * np.pi * t * (2 * mm + 1) / L)
    Bw = -(2.0 / L) * np.sin(2 * np.pi * t * (2 * mm + 1) / L)
    return C5, S5, Mre, Mim, Au, Bu, Aw, Bw


def _build_phase1(mode):
    store = _dt.bfloat16

    nc = bacc.Bacc("TRN2", target_bir_lowering=False, debug=False,
                   num_devices=NCORES)
    # q/k are host pre-shuffled to [NLOC, 128, LB*R]: partition line p holds
    # blocks a=0..7 (x[a*128+p, :]) contiguously -> 1 descriptor per line.
    q_d = nc.dram_tensor("q", [NLOC, 128, LB * R], store,
                         kind="ExternalInput").ap()
    k_d = nc.dram_tensor("k", [NLOC, 128, LB * R], store,
                         kind="ExternalInput").ap()
    cst_d = {}
    for nm in ("c5", "s5", "mre", "mim"):
        cst_d[nm] = nc.dram_tensor(nm, [512, F2], store,
                                   kind="ExternalInput").ap()
    for nm in ("au", "bu", "aw", "bw"):
        cst_d[nm] = nc.dram_tensor(nm, [F2, 512], store,
                                   kind="ExternalInput").ap()
    # u/w halves of the inverse DFT (cols 0..3 = u blocks, 4..7 = w blocks,
    # partition-line layout); host does the final u+-w combine.
    uw_d = nc.dram_tensor("uw", [NLOC, 128, LB * R], _dt.bfloat16,
                          kind="ExternalOutput").ap()
    # per-group row-sums of P: cols 0..3 = re (E0,E1,O0,O1), 4..7 = im
    pacc_d = nc.dram_tensor("pacc", [NLOC, 128, 8], _dt.float32,
                            kind="ExternalOutput").ap()

    def mm(ps, lhsT, rhs, start, stop):
        nc.tensor.matmul(ps, lhsT, rhs, start=start, stop=stop)

    with tile.TileContext(nc) as tc:
        with tc.tile_pool(name="const", bufs=1) as cp, \
             tc.tile_pool(name="qk", bufs=4) as qk, \
             tc.tile_pool(name="ed", bufs=8) as edp, \
             tc.tile_pool(name="pp", bufs=8) as pp, \
             tc.tile_pool(name="tmp", bufs=3) as tp, \
             tc.tile_pool(name="out", bufs=2) as op, \
             tc.tile_pool(name="acc", bufs=5) as accp, \
             tc.tile_pool(name="ps", bufs=4, space="PSUM") as psf, \
             tc.tile_pool(name="psi", bufs=4, space="PSUM") as psi:

            # Batched const loads: one DMA trigger per matrix.
            # fwd mats [512, F2] -> [128, 4*F2]; inv mats [F2, 512] -> [128, 2*512]
            # All input loads issued UPFRONT so no load trigger ever queues
            # behind a store trigger waiting on compute (head-of-line block).
            cmats = {}
            qts, kts = [], []
            q0 = qk.tile([128, LB * R], store, tag="q")
            nc.sync.dma_start(q0[:], q_d[0])
            k0 = qk.tile([128, LB * R], store, tag="k")
            nc.scalar.dma_start(k0[:], k_d[0])
            qts.append(q0)
            kts.append(k0)
            for i, nm in enumerate(("c5", "s5", "mre", "mim")):
                t = cp.tile([128, HB * F2], store, tag=nm)
                (nc.sync if i % 2 else nc.scalar).dma_start(
                    t[:], cst_d[nm].rearrange("(j p) c -> p j c", p=128))
                cmats[nm] = t
            for i, nm in enumerate(("au", "bu", "aw", "bw")):
                t = cp.tile([128, 2 * 512], store, tag=nm)
                (nc.scalar if i % 2 else nc.sync).dma_start(
                    t[:], cst_d[nm].rearrange("(g p) c -> p g c", p=128))
                cmats[nm] = t
            def cslice(nm, j, mb, w):
                # fwd stationary block j: rows 128 (contraction), 128 cols
                # out of the per-block width w (128 or F2)
                return cmats[nm][:, j * w + mb * 128:j * w + mb * 128 + 128]

            def islice(nm, gb, tb):
                return cmats[nm][:, gb * 512 + tb * 128:gb * 512 + tb * 128 + 128]

            for n in range(NLOC):
                if n == 0:
                    qt, kt = qts[0], kts[0]
                else:
                    qt = qk.tile([128, LB * R], store, tag="q")
                    nc.sync.dma_start(qt[:], q_d[n])
                    kt = qk.tile([128, LB * R], store, tag="k")
                    nc.scalar.dma_start(kt[:], k_d[n])

                # radix-2 butterflies: one [128, R] op per (tensor, j) block,
                # DVE (2x bf16) takes 10, GpSimd 6.
                eq, dq, ek, dk = [], [], [], []
                plan = (("eq", eq, qt, "tensor_add", (0, 0, 1, 0)),
                        ("dq", dq, qt, "tensor_sub", (0, 1, 0, 1)),
                        ("ek", ek, kt, "tensor_add", (0, 0, 0, 0)),
                        ("dk", dk, kt, "tensor_sub", (1, 1, 1, 1)))
                for tag, lst, xt, fn, eng in plan:
                    for j in range(HB):
                        t = edp.tile([128, R], store, tag=tag)
                        e = nc.gpsimd if eng[j] else nc.vector
                        getattr(e, fn)(t[:], xt[:, ts(j, R)],
                                       xt[:, ts(j + 4, R)])
                        lst.append(t)

                acc = accp.tile([128, 8], _dt.float32, tag="acc")
                pre_sb, pim_sb = [], []
                groups = [("c5", "s5", eq, ek, 0, F2, HB),
                          ("c5", "s5", eq, ek, 1, F2, HB),
                          ("mre", "mim", dq, dk, 0, F2, HB),
                          ("mre", "mim", dq, dk, 1, F2, HB)]
                for gi, (ma, mb_, xq, xk, mb, cw, nj) in enumerate(groups):
                    ps_qre = psf.tile([128, R], _dt.float32, tag="fwd")
                    ps_qim = psf.tile([128, R], _dt.float32, tag="fwd")
                    ps_kre = psf.tile([128, R], _dt.float32, tag="fwd")
                    ps_kim = psf.tile([128, R], _dt.float32, tag="fwd")
                    for j in range(nj):
                        mm(ps_qre[:], cslice(ma, j, mb, cw), xq[j][:],
                           j == 0, j == nj - 1)
                    for j in range(nj):
                        mm(ps_kre[:], cslice(ma, j, mb, cw), xk[j][:],
                           j == 0, j == nj - 1)
                    for j in range(nj):
                        mm(ps_qim[:], cslice(mb_, j, mb, cw), xq[j][:],
                           j == 0, j == nj - 1)
                    for j in range(nj):
                        mm(ps_kim[:], cslice(mb_, j, mb, cw), xk[j][:],
                           j == 0, j == nj - 1)

                    # stage Q/K to bf16 SBUF (DVE 2x mode for the muls);
                    # copies split DVE/ACT to balance engine load
                    qre = tp.tile([128, R], store, tag="qre")
                    qim = tp.tile([128, R], store, tag="qim")
                    kre = tp.tile([128, R], store, tag="kre")
                    kim = tp.tile([128, R], store, tag="kim")
                    nc.scalar.mul(qre[:], ps_qre[:], 1.0)
                    nc.scalar.mul(qim[:], ps_qim[:], 1.0)
                    nc.scalar.mul(kre[:], ps_kre[:], 1.0)
                    nc.scalar.mul(kim[:], ps_kim[:], 1.0)
                    t1 = tp.tile([128, R], store, tag="t1")
                    t2 = tp.tile([128, R], store, tag="t2")
                    nc.vector.tensor_mul(t1[:], qre[:], kre[:])
                    nc.vector.tensor_mul(t2[:], qim[:], kim[:])
                    pre = pp.tile([128, R], store, tag="pre")
                    nc.vector.scalar_tensor_tensor(
                        pre[:], t1[:], 1.0, t2[:],
                        op0=mybir.AluOpType.mult, op1=mybir.AluOpType.add,
                        accum_out=acc[:, gi:gi + 1])
                    t3 = tp.tile([128, R], store, tag="t3")
                    t4 = tp.tile([128, R], store, tag="t4")
                    nc.vector.tensor_mul(t3[:], qim[:], kre[:])
                    nc.vector.tensor_mul(t4[:], qre[:], kim[:])
                    pim = pp.tile([128, R], store, tag="pim")
                    nc.vector.scalar_tensor_tensor(
                        pim[:], t3[:], 1.0, t4[:],
                        op0=mybir.AluOpType.mult, op1=mybir.AluOpType.subtract,
                        accum_out=acc[:, 4 + gi:5 + gi])
                    if gi == 0:
                        # slot 0 packs DC (re) / Nyquist (im): overwrite with
                        # pure products and patch the two accum elements
                        nc.vector.tensor_copy(pre[0:1, :], t1[0:1, :])
                        nc.vector.tensor_copy(pim[0:1, :], t2[0:1, :])
                        nc.vector.tensor_reduce(
                            acc[0:1, 0:1], t1[0:1, :],
                            axis=mybir.AxisListType.X, op=mybir.AluOpType.add)
                        nc.vector.tensor_reduce(
                            acc[0:1, 4:5], t2[0:1, :],
                            axis=mybir.AxisListType.X, op=mybir.AluOpType.add)
                    pre_sb.append(pre)
                    pim_sb.append(pim)

                uw = op.tile([128, LB * R], store, tag="uw")
                for tb in range(HB):
                    ps_u = psi.tile([128, R], _dt.float32, tag="inv")
                    ps_w = psi.tile([128, R], _dt.float32, tag="inv")
                    for gb in range(2):
                        mm(ps_u[:], islice("au", gb, tb),
                           pre_sb[gb][:], gb == 0, False)
                        mm(ps_u[:], islice("bu", gb, tb),
                           pim_sb[gb][:], False, gb == 1)
                    for gb in range(2):
                        mm(ps_w[:], islice("aw", gb, tb),
                           pre_sb[2 + gb][:], gb == 0, False)
                        mm(ps_w[:], islice("bw", gb, tb),
                           pim_sb[2 + gb][:], False, gb == 1)
                    nc.scalar.mul(uw[:, ts(tb, R)], ps_u[:], 1.0)
                    nc.scalar.mul(uw[:, ts(tb + HB, R)], ps_w[:], 1.0)
                nc.sync.dma_start(uw_d[n], uw[:])
                nc.scalar.dma_start(pacc_d[n][:], acc[:])
    nc.compile()
    return nc

def _build_phase2(entries):
    """entries: per output block b, list of (src_block, seg_idx); seg_idx
    indexes the g stationaries tensor (NLOC, NSEG, 128, 128)."""
    nseg = max(si for segs in entries for _, si in segs) + 1
    nc = bacc.Bacc("TRN2", target_bir_lowering=False, debug=False,
                   num_devices=NCORES)
    # v host pre-shuffled to [NLOC, 128, LB*R] (block a at cols a*R).
    v_d = nc.dram_tensor("v", [NLOC, 128, LB * R], _dt.bfloat16,
                         kind="ExternalInput").ap()
    # g is host-packed as (NLOC, 128, nseg*128): one contiguous DMA per n;
    # stationary si is the [:, si*128:(si+1)*128] slice.
    g_d = nc.dram_tensor("g", [NLOC, 128, nseg * 128], _dt.bfloat16,
                         kind="ExternalInput").ap()
    out_d = nc.dram_tensor("out", [NLOC, 128, LB * R], _dt.bfloat16,
                           kind="ExternalOutput").ap()

    with tile.TileContext(nc) as tc:
        with tc.tile_pool(name="v", bufs=NLOC) as vp, \
             tc.tile_pool(name="g", bufs=NLOC) as gp, \
             tc.tile_pool(name="o", bufs=3) as op, \
             tc.tile_pool(name="ps", bufs=8, space="PSUM") as psp:
            # One batched DMA per tensor: v[n] -> [128, LB*R], g[n] whole.
            v_sb, g_sb = [], []
            for n in range(NLOC):
                t = vp.tile([128, LB * R], _dt.bfloat16, tag="v")
                (nc.scalar if n % 2 else nc.sync).dma_start(t[:], v_d[n])
                v_sb.append(t)
                tg = gp.tile([128, nseg * 128], _dt.bfloat16, tag="g")
                (nc.sync if n % 2 else nc.scalar).dma_start(tg[:], g_d[n][:])
                g_sb.append(tg)
            for n in range(NLOC):
                ot = op.tile([128, LB * R], _dt.bfloat16, tag="o")
                for b in range(LB):
                    segs = entries[b]
                    ps = psp.tile([128, R], _dt.float32, tag="ps")
                    for i, (a, si) in enumerate(segs):
                        nc.tensor.matmul(ps[:], g_sb[n][:, ts(si, 128)],
                                         v_sb[n][:, ts(a, R)],
                                         start=(i == 0),
                                         stop=(i == len(segs) - 1))
                    if b % 2:
                        nc.scalar.mul(ot[:, ts(b, R)], ps[:], 1.0)
                    else:
                        nc.vector.tensor_copy(ot[:, ts(b, R)], ps[:])
                nc.sync.dma_start(out_d[n], ot[:])
    nc.compile()
    return nc


_P1_CACHE = {}


def _phase1_nc(mode):
    if mode not in _P1_CACHE:
        _P1_CACHE[mode] = _build_phase1(mode)
    return _P1_CACHE[mode]


def _run(nc, in_maps, phase):
    res = run_bass_kernel_spmd(nc, in_maps, core_ids=list(range(NCORES)),
                               trace=TRACE[0])
    if TRACE[0]:
        LAST_EXEC_NS[phase] = res.exec_time_ns
    return res.results


def kernel(queries, keys, values):
    queries = np.ascontiguousarray(np.asarray(queries, dtype=np.float32))
    keys = np.ascontiguousarray(np.asarray(keys, dtype=np.float32))
    values = np.ascontiguousarray(np.asarray(values, dtype=np.float32))

    mode = P1_MODE
    store_np = ml_dtypes.bfloat16
    C5, S5, Mre, Mim, Au, Bu, Aw, Bw = _dft_mats()
    consts = {
        "c5": C5, "s5": S5, "mre": Mre, "mim": Mim,
        "au": Au, "bu": Bu, "aw": Aw, "bw": Bw,
    }
    consts = {k: np.ascontiguousarray(v.astype(np.float32)).astype(store_np)
              for k, v in consts.items()}

    # shuffle to [N, 128, LB, R]: SBUF partition line p = blocks a contiguous
    def shuf(x):
        return np.ascontiguousarray(
            x.reshape(N, LB, 128, R).transpose(0, 2, 1, 3)
        ).reshape(N, 128, LB * R)

    q3 = shuf(queries.reshape(N, L, R).astype(store_np))
    k3 = shuf(keys.reshape(N, L, R).astype(store_np))
    v3 = shuf(values.reshape(N, L, R).astype(ml_dtypes.bfloat16))

    nc1 = _phase1_nc(mode)
    in_maps = []
    for c in range(NCORES):
        sl = slice(c * NLOC, (c + 1) * NLOC)
        in_maps.append({
            "q": q3[sl],
            "k": k3[sl],
            **consts,
        })
    res1 = _run(nc1, in_maps, 0)

    uw = np.concatenate([np.asarray(r["uw"], np.float32) for r in res1],
                        axis=0)                        # (N, 128, 8*R)
    uw = uw.reshape(N, 128, LB, R).transpose(0, 2, 1, 3)  # (N, 8, 128, R)
    u = uw[:, 0:HB].reshape(N, 512, R)
    wv = uw[:, HB:LB].reshape(N, 512, R)
    corr = np.concatenate([u + wv, u - wv], axis=1)           # (N, L, R) f32
    pacc = np.concatenate([r["pacc"] for r in res1], axis=0)  # (N, 128, 8)
    # reconstruct mean over (H,E) from per-group P row-sums (host irfft on
    # a 512-vector per batch item)
    pacc = pacc.astype(np.float64)
    per_ = pacc[:, :, 0:2].transpose(0, 2, 1).reshape(N, 256)   # Pe_re sums
    por_ = pacc[:, :, 2:4].transpose(0, 2, 1).reshape(N, 256)   # Po_re
    pei_ = pacc[:, :, 4:6].transpose(0, 2, 1).reshape(N, 256)   # Pe_im
    poi_ = pacc[:, :, 6:8].transpose(0, 2, 1).reshape(N, 256)   # Po_im
    um = per_ @ Au + pei_ @ Bu
    wm = por_ @ Aw + poi_ @ Bw
    mean = np.concatenate([um + wm, um - wm], axis=1) / R       # (N, L)

    g = mean.mean(axis=0)
    idx = np.argsort(-g, kind="stable")[:TOPK]
    w = mean[:, idx]
    e = np.exp(w - w.max(axis=1, keepdims=True))
    w = (e / e.sum(axis=1, keepdims=True)).astype(np.float32)  # (N, TOPK)

    # phase-2 stationaries: out[b*128+j] += w_k * v[(b*128+j+idx_k) mod L]
    # merged per (b, src_block); matrix content is b-independent, so dedup
    # identical segment sets across b.
    seg_of = {}
    pat = []
    entries = [[] for _ in range(LB)]
    for b in range(LB):
        acc = {}
        for kk in range(TOPK):
            sh = int(idx[kk])
            r = sh % 128
            a = ((b * 128 + sh) // 128) % LB
            acc.setdefault(a, []).append(("d1", r, kk))
            if r > 0:
                acc.setdefault((a + 1) % LB, []).append(("d2", r, kk))
        for a, parts in sorted(acc.items()):
            key = tuple(sorted(parts))
            if key not in seg_of:
                seg_of[key] = len(pat)
                pat.append(parts)
            entries[b].append((a, seg_of[key]))
    nseg = len(pat)
    gmat = np.zeros((NLOC * NCORES, nseg, 128, 128), np.float32)
    jj = np.arange(128)
    for si, parts in enumerate(pat):
        for which, r, kk in parts:
            if which == "d1":
                j = jj[: 128 - r]
                gmat[:, si, j + r, j] += w[:, kk][:, None]
            else:
                j = jj[128 - r:]
                gmat[:, si, j - (128 - r), j] += w[:, kk][:, None]
    # pack (NLOC, nseg, 128, 128) -> (NLOC, 128, nseg*128) for 1-DMA-per-n
    gmat = np.ascontiguousarray(
        gmat.transpose(0, 2, 1, 3).reshape(NLOC * NCORES, 128, nseg * 128)
    ).astype(ml_dtypes.bfloat16)

    nc2 = _build_phase2(entries)
    in_maps2 = []
    for c in range(NCORES):
        sl = slice(c * NLOC, (c + 1) * NLOC)
        in_maps2.append({
            "v": v3[sl],
            "g": gmat[sl],
        })
    res2 = _run(nc2, in_maps2, 1)
    out = np.concatenate([np.asarray(r["out"], dtype=np.float32)
                          for r in res2], axis=0)   # (N, 128, LB*R) shuffled
    # un-shuffle: [N, 128, LB, R] -> [N, LB, 128, R] -> (N, L, R)
    out = out.reshape(N, 128, LB, R).transpose(0, 2, 1, 3).reshape(N, L, R)

    out_full = np.ascontiguousarray(out).reshape(N, L, H, E)
    corr_full = corr.reshape(N, L, H, E).astype(np.float32)
    return out_full, corr_full

